# revision 1
# baseline (speedup 1.0000x reference)
"""Causal self-attention on 8 Trainium2 NeuronCores.

Sharding: core c = 2*b + g handles batch b (of 4) and head-group g (8 of 16
heads). Each core computes the qkv projection for its head slice, causal
attention for its 8 heads, and the output projection against its 512-row
slice of w_proj. The two half-projections per batch are summed on the host
(plus b_proj).

Per-core layout:
  Q^T, K^T [128, 2048] feature-major per pair of heads (rotating pool tiles)
  V [2048, 768] token-major as [V_even | ones | V_odd] per head pair: the AV
    stationary [V_h | ones] (or [ones | V_h]) yields O^T rows plus the
    softmax denominator replicated over the other 64 partitions in one shot.
  S^T = K @ Q^T per (head, 512-wide q super-block): no transposes anywhere,
    masking is one additive -1e9 region per diagonal-block chunk before exp.
  Attention for pair p is interleaved with the Q/K projection of pair p+1 so
  the PE always has dependency-free matmuls while ACT computes exp.
All matmuls run in float32r (TF32-like, ~1.5e-4 rel err).
"""

import sys

sys.path.insert(0, "/opt/trn_rl_repo")

import numpy as np

import concourse.bass as bass
import concourse.mybir as mybir
import concourse.tile as tile
from concourse import bacc
from concourse.bass import AP

F32 = mybir.dt.float32
F32R = mybir.dt.float32r
AF = mybir.ActivationFunctionType

N_CORES = 8
T = 2048
C = 1024
D = 64
P = 128
NT = T // P     # 16 token chunks
NS = 4          # q super-blocks of 512
CK = C // P     # 8 contraction chunks
NEG = -1e9


def build_nc(repeat: int = 1, timing: bool = False):
    nc = bacc.Bacc("TRN2", target_bir_lowering=False, debug=False)

    xt = nc.dram_tensor("xt", [C, T], F32, kind="ExternalInput").ap()
    wq = nc.dram_tensor("wq", [C, 512], F32, kind="ExternalInput").ap()
    wk = nc.dram_tensor("wk", [C, 512], F32, kind="ExternalInput").ap()
    wv = nc.dram_tensor("wv", [C, 512], F32, kind="ExternalInput").ap()
    wp = nc.dram_tensor("wp", [512, C], F32, kind="ExternalInput").ap()
    bq = nc.dram_tensor("bq", [512, 1], F32, kind="ExternalInput").ap()
    bk = nc.dram_tensor("bk", [512, 1], F32, kind="ExternalInput").ap()
    bv = nc.dram_tensor("bv", [1, 512], F32, kind="ExternalInput").ap()
    # mask[:, r*512:(r+1)*512] is the additive mask for the diagonal-super
    # chunk with within-super row index r (cols 0..r*128 fully -1e9, block r
    # strictly-lower-triangle -1e9, rest 0)
    mask = nc.dram_tensor("mask", [P, 4 * 512], F32, kind="ExternalInput").ap()
    if timing:
        out = nc.dram_tensor("out", [T, C], F32).ap()
        probe = nc.dram_tensor("probe", [P, 4], F32, kind="ExternalOutput").ap()
    else:
        out = nc.dram_tensor("out", [T, C], F32, kind="ExternalOutput").ap()
        probe = None

    with tile.TileContext(nc) as tc:
        with tc.tile_pool(name="persist", bufs=1) as pp:
            V = [pp.tile([P, 768], F32R, name=f"v{j}", tag=f"v{j}") for j in range(NT)]
            mask_t = pp.tile([P, 4 * 512], F32, tag="mask_t")
            bq_t = [pp.tile([P, 1], F32, name=f"bq{m}", tag=f"bq{m}") for m in range(4)]
            bk_t = [pp.tile([P, 1], F32, name=f"bk{m}", tag=f"bk{m}") for m in range(4)]
            bv_t = pp.tile([1, 512], F32R, tag="bv")
            ones_t = pp.tile([1, P], F32R, tag="ones")

            nc.sync.dma_start(mask_t[:], mask[:])
            for m in range(4):
                nc.sync.dma_start(bq_t[m][:], bq[m * P:(m + 1) * P, :])
                nc.sync.dma_start(bk_t[m][:], bk[m * P:(m + 1) * P, :])
            nc.sync.dma_start(bv_t[:], bv[:].bitcast(F32R))
            nc.vector.memset(ones_t[:].bitcast(F32), 1.0)
            for j in range(NT):
                nc.vector.memset(V[j][:].bitcast(F32), 1.0)

            chain = repeat > 1
            tok = None
            for _rep in range(repeat):
                rep_pool_cm = tc.tile_pool(name=f"rp{_rep}", bufs=1)
                rp = rep_pool_cm.__enter__()
                if chain and _rep > 0:
                    # serialize iterations (timing builds): biases depend on
                    # 0 x previous iteration's output.
                    bq_e, bk_e = [], []
                    for m in range(4):
                        t1 = rp.tile([P, 1], F32, name=f"bqe{_rep}_{m}", tag=f"bqe{_rep}_{m}")
                        nc.vector.tensor_add(t1[:], bq_t[m][:], tok[:])
                        bq_e.append(t1)
                        t2 = rp.tile([P, 1], F32, name=f"bke{_rep}_{m}", tag=f"bke{_rep}_{m}")
                        nc.vector.tensor_add(t2[:], bk_t[m][:], tok[:])
                        bk_e.append(t2)
                    bv_e = rp.tile([1, 512], F32R, name=f"bve{_rep}", tag=f"bve{_rep}")
                    nc.vector.tensor_scalar_add(bv_e[:], bv_t[:], tok[0:1, 0:1])
                else:
                    bq_e, bk_e, bv_e = bq_t, bk_t, bv_t

                # ---- phase C: V projection (V = x @ Wv + bias row) ----
                with tc.tile_pool(name="wvt", bufs=1) as wvpool, \
                     tc.tile_pool(name="xtv", bufs=10) as xpool, \
                     tc.tile_pool(name="vps", bufs=6, space="PSUM") as vps:
                    WV = [wvpool.tile([P, 512], F32R, name=f"wv{k}", tag=f"wv{k}") for k in range(CK)]
                    for k in range(CK):
                        nc.sync.dma_start(WV[k][:], wv[k * P:(k + 1) * P, :].bitcast(F32R))
                    for n in range(4):
                        xts = []
                        for k in range(CK):
                            xtile = xpool.tile([P, 512], F32R, tag="xt")
                            nc.sync.dma_start(
                                xtile[:],
                                xt[k * P:(k + 1) * P, n * 512:(n + 1) * 512].bitcast(F32R),
                            )
                            xts.append(xtile)
                        for i in range(4):
                            ps = vps.tile([P, 512], F32, tag="ps")
                            for k in range(CK):
                                nc.tensor.matmul(
                                    ps[:], xts[k][:, i * P:(i + 1) * P], WV[k][:],
                                    start=(k == 0), stop=False,
                                )
                            nc.tensor.matmul(ps[:], ones_t[:], bv_e[:],
                                             start=False, stop=True)
                            vap = V[n * 4 + i][:]
                            part = list(vap.ap)[0]
                            psap = ps[:]
                            pspart = list(psap.ap)[0]
                            nc.vector.tensor_copy(
                                AP(vap.tensor, vap.offset, [part, (192, 4), (1, 64)]),
                                AP(psap.tensor, psap.offset, [pspart, (128, 4), (1, 64)]))
                            nc.vector.tensor_copy(
                                AP(vap.tensor, vap.offset + 128, [part, (192, 4), (1, 64)]),
                                AP(psap.tensor, psap.offset + 64, [pspart, (128, 4), (1, 64)]))

                # ---- interleaved B (Q/K projection per pair) and D ----
                bd = tc.tile_pool(name="bd", bufs=1)
                bdp = bd.__enter__()
                WQ = [bdp.tile([P, 512], F32R, name=f"wq{k}", tag=f"wq{k}") for k in range(CK)]
                WK = [bdp.tile([P, 512], F32R, name=f"wk{k}", tag=f"wk{k}") for k in range(CK)]
                for k in range(CK):
                    nc.sync.dma_start(WQ[k][:], wq[k * P:(k + 1) * P, :].bitcast(F32R))
                    nc.sync.dma_start(WK[k][:], wk[k * P:(k + 1) * P, :].bitcast(F32R))
                YT = [bdp.tile([P, T], F32R, name=f"yt{p}", tag=f"yt{p}") for p in range(4)]

                xqpool = tc.tile_pool(name="xtq", bufs=10)
                xqp = xqpool.__enter__()
                qtpool = tc.tile_pool(name="qtp", bufs=2)
                qtp = qtpool.__enter__()
                bdps = tc.tile_pool(name="bdps", bufs=2, space="PSUM")
                bdpsp = bdps.__enter__()
                spool_cm = tc.tile_pool(name="spsum", bufs=2, space="PSUM")
                spool = spool_cm.__enter__()
                avpool_cm = tc.tile_pool(name="avps", bufs=2, space="PSUM")
                avpool = avpool_cm.__enter__()
                ptpool_cm = tc.tile_pool(name="pt", bufs=2)
                ptpool = ptpool_cm.__enter__()
                lpool_cm = tc.tile_pool(name="lrec", bufs=1)
                lpool = lpool_cm.__enter__()

                QTs = [None] * 4
                KTs = [None] * 4

                def b_step(p, n):
                    # Q^T/K^T chunk for pair p, token-slice n
                    if n == 0:
                        QTs[p] = qtp.tile([P, T], F32R, name=f"qtt{p}", tag="qtt")
                        KTs[p] = qtp.tile([P, T], F32R, name=f"ktt{p}", tag="ktt")
                    xts = []
                    for k in range(CK):
                        xtile = xqp.tile([P, 512], F32R, tag="xt2")
                        nc.sync.dma_start(
                            xtile[:],
                            xt[k * P:(k + 1) * P, n * 512:(n + 1) * 512].bitcast(F32R),
                        )
                        xts.append(xtile)
                    ps = bdpsp.tile([P, 512], F32, tag="bps")
                    for k in range(CK):
                        nc.tensor.matmul(
                            ps[:], WQ[k][:, p * P:(p + 1) * P], xts[k][:],
                            start=(k == 0), stop=(k == CK - 1),
                        )
                    nc.vector.tensor_scalar_add(
                        QTs[p][:, n * 512:(n + 1) * 512], ps[:], bq_e[p][:])
                    ps = bdpsp.tile([P, 512], F32, tag="bps")
                    for k in range(CK):
                        nc.tensor.matmul(
                            ps[:], WK[k][:, p * P:(p + 1) * P], xts[k][:],
                            start=(k == 0), stop=(k == CK - 1),
                        )
                    nc.vector.tensor_scalar_add(
                        KTs[p][:, n * 512:(n + 1) * 512], ps[:], bk_e[p][:])

                def v_stat_ap(j: int, h: int) -> AP:
                    # [V_even|ones|V_odd] per pair: even h -> [V_h | ones]
                    # (O^T rows 0:64, denom rows 64:128); odd h -> [ones | V_h].
                    e = h // 2
                    start = 192 * e + (64 if h % 2 else 0)
                    return V[j][:, start:start + 128]

                def d_group(p, hh, s):
                    h = 2 * p + hh
                    prow = hh * 64       # O^T partition rows
                    lrow = 64 - prow     # denominator partition rows
                    QT, KT = QTs[p], KTs[p]
                    nch = 4 * s + 4
                    npairs = nch // 2
                    av = avpool.tile([P, 512], F32, tag="av")
                    pts = []

                    def emit_s(jj):
                        sp = spool.tile([P, 1024], F32, tag="sp")
                        for cc in range(2):
                            j = 2 * jj + cc
                            nc.tensor.matmul(
                                sp[:, cc * 512:(cc + 1) * 512],
                                KT[prow:prow + 64, j * P:(j + 1) * P],
                                QT[prow:prow + 64, s * 512:(s + 1) * 512],
                                start=True, stop=True,
                            )
                            r = j - 4 * s
                            if 0 <= r <= 3:
                                # one additive mask covering the whole
                                # invalid region of this chunk
                                w = (r + 1) * P
                                nc.vector.tensor_add(
                                    sp[:, cc * 512: cc * 512 + w],
                                    sp[:, cc * 512: cc * 512 + w],
                                    mask_t[:, r * 512: r * 512 + w],
                                )
                        pt = ptpool.tile([P, 1024], F32R, tag="pt")
                        nc.scalar.activation(pt[:], sp[:], AF.Exp, scale=0.125)
                        pts.append(pt)

                    def emit_av(jj):
                        pt = pts[jj]
                        for cc in range(2):
                            j = 2 * jj + cc
                            nc.tensor.matmul(
                                av[:], v_stat_ap(j, h),
                                pt[:, cc * 512:(cc + 1) * 512],
                                start=(j == 0), stop=(j == nch - 1),
                            )

                    emit_s(0)
                    for jj in range(1, npairs):
                        emit_s(jj)
                        emit_av(jj - 1)
                    emit_av(npairs - 1)

                    # normalization: denom rows -> SBUF (same partitions),
                    # DMA-shift to O^T's partitions, reciprocal + multiply.
                    lt = lpool.tile([P, 512], F32, tag="lt")
                    nc.vector.tensor_copy(
                        lt[lrow:lrow + 64, :], av[lrow:lrow + 64, :])
                    lt2 = lpool.tile([P, 512], F32, tag="lt2")
                    nc.sync.dma_start(
                        lt2[prow:prow + 64, :], lt[lrow:lrow + 64, :])
                    rec = lpool.tile([P, 512], F32, tag="rec")
                    nc.vector.reciprocal(
                        rec[prow:prow + 64, :], lt2[prow:prow + 64, :])
                    nc.vector.tensor_mul(
                        YT[p][prow:prow + 64, s * 512:(s + 1) * 512],
                        av[prow:prow + 64, :],
                        rec[prow:prow + 64, :],
                    )

                # emission: B(0) fully, then for each pair interleave its 8
                # attention groups with the next pair's 4 projection steps.
                for n in range(4):
                    b_step(0, n)
                for p in range(4):
                    groups = [(p, hh, s) for hh in range(2) for s in range(NS)]
                    bsteps = [(p + 1, n) for n in range(4)] if p < 3 else []
                    gi = 0
                    for i, g in enumerate(groups):
                        d_group(*g)
                        if i % 2 == 1 and gi < len(bsteps):
                            b_step(*bsteps[gi])
                            gi += 1
                    while gi < len(bsteps):
                        b_step(*bsteps[gi])
                        gi += 1

                lpool_cm.__exit__(None, None, None)
                ptpool_cm.__exit__(None, None, None)
                avpool_cm.__exit__(None, None, None)
                spool_cm.__exit__(None, None, None)
                bdps.__exit__(None, None, None)
                qtpool.__exit__(None, None, None)
                xqpool.__exit__(None, None, None)

                # ---- phase E: output projection ----
                WP = [bdp.tile([P, C], F32R, name=f"wp{k}", tag=f"wp{k}") for k in range(4)]
                for k in range(4):
                    nc.sync.dma_start(WP[k][:], wp[k * P:(k + 1) * P, :].bitcast(F32R))
                with tc.tile_pool(name="projps", bufs=4, space="PSUM") as prpool, \
                     tc.tile_pool(name="ostage", bufs=4) as opool:
                    for m in range(NT):
                        for nn in range(2):
                            ps = prpool.tile([P, 512], F32, tag="pp")
                            for kf in range(4):
                                nc.tensor.matmul(
                                    ps[:],
                                    YT[kf][:, m * P:(m + 1) * P],
                                    WP[kf][:, nn * 512:(nn + 1) * 512],
                                    start=(kf == 0), stop=(kf == 3),
                                )
                            ost = opool.tile([P, 512], F32, tag="ost")
                            nc.vector.tensor_copy(ost[:], ps[:])
                            nc.sync.dma_start(
                                out[m * P:(m + 1) * P, nn * 512:(nn + 1) * 512], ost[:])
                            last_ost = ost
                    if chain:
                        tok = pp.tile([P, 1], F32, name=f"tok{_rep}", tag=f"tok{_rep}")
                        nc.vector.tensor_scalar_mul(tok[:], last_ost[:, 0:1], 0.0)
                    if timing and _rep == repeat - 1:
                        nc.sync.dma_start(probe[:], last_ost[:, 0:4])
                bd.__exit__(None, None, None)
                rep_pool_cm.__exit__(None, None, None)

    nc.compile()
    return nc


_TRI = np.where(
    np.arange(P)[:, None] > np.arange(P)[None, :],
    np.float32(NEG), np.float32(0.0),
).astype(np.float32)


def _build_mask():
    m = np.zeros((P, 4 * 512), dtype=np.float32)
    for r in range(4):
        blk = m[:, r * 512:(r + 1) * 512]
        blk[:, : r * P] = np.float32(NEG)       # fully-masked col blocks
        blk[:, r * P:(r + 1) * P] = _TRI        # diagonal triangle
    return m


_MASK = _build_mask()


def shard_inputs(x, w_attn, b_attn, w_proj, b_proj):
    """Build the per-core input maps."""
    x = np.asarray(x, dtype=np.float32)
    w_attn = np.asarray(w_attn, dtype=np.float32)
    b_attn = np.asarray(b_attn, dtype=np.float32)
    w_proj = np.asarray(w_proj, dtype=np.float32)
    in_maps = []
    for c in range(N_CORES):
        b, g = divmod(c, 2)
        sl = slice(g * 512, (g + 1) * 512)
        in_maps.append({
            "xt": np.ascontiguousarray(x[b].T),
            "wq": np.ascontiguousarray(w_attn[:, g * 512:(g + 1) * 512]),
            "wk": np.ascontiguousarray(w_attn[:, 1024 + g * 512:1024 + (g + 1) * 512]),
            "wv": np.ascontiguousarray(w_attn[:, 2048 + g * 512:2048 + (g + 1) * 512]),
            "wp": np.ascontiguousarray(w_proj[g * 512:(g + 1) * 512, :]),
            "bq": np.ascontiguousarray(b_attn[sl].reshape(512, 1)),
            "bk": np.ascontiguousarray(b_attn[1024 + g * 512:1024 + (g + 1) * 512].reshape(512, 1)),
            "bv": np.ascontiguousarray(b_attn[2048 + g * 512:2048 + (g + 1) * 512].reshape(1, 512)),
            "mask": _MASK,
        })
    return in_maps


def gather_output(results, b_proj):
    b_proj = np.asarray(b_proj, dtype=np.float32)
    out = np.empty((4, T, C), dtype=np.float32)
    for b in range(4):
        out[b] = results[2 * b]["out"] + results[2 * b + 1]["out"] + b_proj
    return out


_NC_CACHE = None


def get_nc():
    global _NC_CACHE
    if _NC_CACHE is None:
        _NC_CACHE = build_nc()
    return _NC_CACHE


def kernel(x, w_attn, b_attn, w_proj, b_proj):
    from concourse.bass_utils import run_bass_kernel_spmd

    in_maps = shard_inputs(x, w_attn, b_attn, w_proj, b_proj)
    nc = get_nc()
    res = run_bass_kernel_spmd(nc, in_maps, list(range(N_CORES)))
    return gather_output(res.results, b_proj)



# revision 2
# speedup vs baseline: 1.0574x; 1.0574x over previous
"""Causal self-attention on 8 Trainium2 NeuronCores — bf16 pipeline.

Sharding: core c = 2*b + g handles batch b (of 4) and head-group g (8 of 16
heads). Each core computes the qkv projection for its head slice, causal
attention for its 8 heads, and the output projection against its 512-row
slice of w_proj. The two half-projections per batch are summed on the host
(plus b_proj).

Key optimizations:
  - All matmul operands bf16 (host-cast inputs); PSUM accumulation f32.
    Halves DMA + SBUF, enables FWL weight loads and DVE 2x modes.
  - x^T resident in SBUF (4 MiB bf16), loaded once — v1 re-loaded it 5x.
  - S^T for the two heads of a pair computed as two concurrent PE row-tiles
    (stationary K^T at base partitions 0/64 -> tile_position (0,0)/(64,0)).
  - Causal mask is multiplicative bf16 on P after exp (DVE 2x) instead of
    additive f32 on PSUM.
  - Output written bf16, upcast + summed on host.
  - Prefetch ordering: x first half -> wv -> x second half -> wq/wk/wp, so
    the PE never waits long for DMA; output projection interleaved into
    pair-3 attention (PSUM freed by closing the projection-psum pool first).

Per-core layout:
  Q^T, K^T [128, 2048] bf16 feature-major per pair (2 heads x 64 feats).
  V [2048, 768] bf16 token-major as [V_even | ones | V_odd] per pair: the AV
  stationary [V_h | ones] (or [ones | V_h]) yields O^T rows plus the softmax
  denominator replicated over the other 64 partitions in one shot.
"""

import sys

sys.path.insert(0, "/opt/trn_rl_repo")

import numpy as np
import ml_dtypes

import concourse.bass as bass
import concourse.mybir as mybir
import concourse.tile as tile
from concourse import bacc
from concourse.bass import AP

F32 = mybir.dt.float32
BF16 = mybir.dt.bfloat16
AF = mybir.ActivationFunctionType
NPBF16 = ml_dtypes.bfloat16

N_CORES = 8
T = 2048
C = 1024
D = 64
P = 128
NT = T // P     # 16 token chunks
NS = 4          # q super-blocks of 512
CK = C // P     # 8 contraction chunks


def build_nc(repeat: int = 1, timing: bool = False):
    nc = bacc.Bacc("TRN2", target_bir_lowering=False, debug=False)

    xt = nc.dram_tensor("xt", [C, T], BF16, kind="ExternalInput").ap()
    wq = nc.dram_tensor("wq", [C, 512], BF16, kind="ExternalInput").ap()
    wk = nc.dram_tensor("wk", [C, 512], BF16, kind="ExternalInput").ap()
    wv = nc.dram_tensor("wv", [C, 512], BF16, kind="ExternalInput").ap()
    wp = nc.dram_tensor("wp", [512, C], BF16, kind="ExternalInput").ap()
    bq = nc.dram_tensor("bq", [512, 1], F32, kind="ExternalInput").ap()
    bk = nc.dram_tensor("bk", [512, 1], F32, kind="ExternalInput").ap()
    bv = nc.dram_tensor("bv", [1, 512], BF16, kind="ExternalInput").ap()
    # maskm[:, r*1024:(r+1)*1024] = [m_r | m_r] (dup for the 2 heads of the
    # pt tile); m_r[k, q] = 1 if q >= 128*r + k else 0 — multiplicative
    # causal mask for diagonal-super chunk with within-super row index r.
    maskm = nc.dram_tensor("maskm", [P, 4 * 1024], BF16, kind="ExternalInput").ap()
    if timing:
        out = nc.dram_tensor("out", [T, C], BF16).ap()
        probe = nc.dram_tensor("probe", [P, 4], BF16, kind="ExternalOutput").ap()
    else:
        out = nc.dram_tensor("out", [T, C], BF16, kind="ExternalOutput").ap()
        probe = None

    with tile.TileContext(nc) as tc:
        with tc.tile_pool(name="persist", bufs=1) as pp:
            XT = [pp.tile([P, T], BF16, name=f"x{k}", tag=f"x{k}") for k in range(CK)]
            V = [pp.tile([P, 768], BF16, name=f"v{j}", tag=f"v{j}") for j in range(NT)]
            mask_t = pp.tile([P, 4 * 1024], BF16, tag="mask_t")
            bq_t = [pp.tile([P, 1], F32, name=f"bq{m}", tag=f"bq{m}") for m in range(4)]
            bk_t = [pp.tile([P, 1], F32, name=f"bk{m}", tag=f"bk{m}") for m in range(4)]
            bv_t = pp.tile([1, 512], BF16, tag="bv")
            ones_t = pp.tile([1, P], BF16, tag="ones")

            nc.vector.memset(ones_t[:], 1.0)
            for j in range(NT):
                nc.vector.memset(V[j][:], 1.0)
            for m in range(4):
                nc.sync.dma_start(bq_t[m][:], bq[m * P:(m + 1) * P, :])
                nc.sync.dma_start(bk_t[m][:], bk[m * P:(m + 1) * P, :])
            nc.sync.dma_start(bv_t[:], bv[:])
            nc.sync.dma_start(mask_t[:], maskm[:])

            chain = repeat > 1
            tok = None
            for _rep in range(repeat):
                rep_pool_cm = tc.tile_pool(name=f"rp{_rep}", bufs=1)
                rp = rep_pool_cm.__enter__()
                if chain and _rep > 0:
                    # serialize iterations (timing builds): biases depend on
                    # 0 x previous iteration's output.
                    bq_e, bk_e = [], []
                    for m in range(4):
                        t1 = rp.tile([P, 1], F32, name=f"bqe{_rep}_{m}", tag=f"bqe{_rep}_{m}")
                        nc.vector.tensor_add(t1[:], bq_t[m][:], tok[:])
                        bq_e.append(t1)
                        t2 = rp.tile([P, 1], F32, name=f"bke{_rep}_{m}", tag=f"bke{_rep}_{m}")
                        nc.vector.tensor_add(t2[:], bk_t[m][:], tok[:])
                        bk_e.append(t2)
                    bv_e = rp.tile([1, 512], BF16, name=f"bve{_rep}", tag=f"bve{_rep}")
                    nc.vector.tensor_scalar_add(bv_e[:], bv_t[:], tok[0:1, 0:1])
                else:
                    bq_e, bk_e, bv_e = bq_t, bk_t, bv_t

                # weight + x pools live for the whole iteration
                bd = tc.tile_pool(name=f"bd{_rep}", bufs=1)
                bdp = bd.__enter__()

                # prefetch order: wv -> x in column slices (first slices gate
                # the first V-projection chunks) -> wq/wk -> wp
                WV = [bdp.tile([P, 512], BF16, name=f"wv{k}", tag=f"wv{k}") for k in range(CK)]
                for k in range(CK):
                    nc.sync.dma_start(WV[k][:], wv[k * P:(k + 1) * P, :])
                for c0, c1 in ((0, 512), (512, 1024), (1024, 2048)):
                    for k in range(CK):
                        nc.sync.dma_start(XT[k][:, c0:c1],
                                          xt[k * P:(k + 1) * P, c0:c1])
                WQ = [bdp.tile([P, 512], BF16, name=f"wq{k}", tag=f"wq{k}") for k in range(CK)]
                WK = [bdp.tile([P, 512], BF16, name=f"wk{k}", tag=f"wk{k}") for k in range(CK)]
                for k in range(CK):
                    nc.sync.dma_start(WQ[k][:], wq[k * P:(k + 1) * P, :])
                    nc.sync.dma_start(WK[k][:], wk[k * P:(k + 1) * P, :])
                WP = [bdp.tile([P, C], BF16, name=f"wp{k}", tag=f"wp{k}") for k in range(4)]
                for k in range(4):
                    nc.sync.dma_start(WP[k][:], wp[k * P:(k + 1) * P, :])
                YT = [bdp.tile([P, T], BF16, name=f"yt{p}", tag=f"yt{p}") for p in range(4)]

                # ---- phase V: V projection (V = x @ Wv + bias row) ----
                with tc.tile_pool(name=f"vps{_rep}", bufs=4, space="PSUM") as vps:
                    for j in range(NT):
                        ps = vps.tile([P, 512], F32, tag="ps")
                        for k in range(CK):
                            nc.tensor.matmul(
                                ps[:], XT[k][:, j * P:(j + 1) * P], WV[k][:],
                                start=(k == 0), stop=False,
                            )
                        nc.tensor.matmul(ps[:], ones_t[:], bv_e[:],
                                         start=False, stop=True)
                        # interleave [V_even | ones | V_odd]: psum cols
                        # p*128..p*128+64 -> V cols 192p..192p+64, and
                        # psum cols p*128+64.. -> V cols 192p+128..192p+192
                        vap = V[j][:]
                        part = list(vap.ap)[0]
                        psap = ps[:]
                        pspart = list(psap.ap)[0]
                        nc.vector.tensor_copy(
                            AP(vap.tensor, vap.offset, [part, (192, 4), (1, 64)]),
                            AP(psap.tensor, psap.offset, [pspart, (128, 4), (1, 64)]))
                        nc.vector.tensor_copy(
                            AP(vap.tensor, vap.offset + 128, [part, (192, 4), (1, 64)]),
                            AP(psap.tensor, psap.offset + 64, [pspart, (128, 4), (1, 64)]))

                # ---- interleaved B (Q/K projection per pair) and D ----
                qtpool = tc.tile_pool(name="qtp", bufs=2)
                qtp = qtpool.__enter__()
                spool_cm = tc.tile_pool(name="spsum", bufs=2, space="PSUM")
                spool = spool_cm.__enter__()
                avpool_cm = tc.tile_pool(name="avps", bufs=1, space="PSUM")
                avpool = avpool_cm.__enter__()
                ptpool_cm = tc.tile_pool(name="pt", bufs=3)
                ptpool = ptpool_cm.__enter__()
                lpool_cm = tc.tile_pool(name="lrec", bufs=2)
                lpool = lpool_cm.__enter__()
                # opened last so it can be closed (freeing 2 PSUM banks)
                # before the output-projection pool opens
                bdps = tc.tile_pool(name="bdps", bufs=2, space="PSUM")
                bdpsp = bdps.__enter__()

                QTs = [None] * 4
                KTs = [None] * 4

                def b_step(p, n):
                    # Q^T/K^T chunk for pair p, token-slice n
                    if n == 0:
                        QTs[p] = qtp.tile([P, T], BF16, name=f"qtt{p}", tag="qtt")
                        KTs[p] = qtp.tile([P, T], BF16, name=f"ktt{p}", tag="ktt")
                    ps = bdpsp.tile([P, 512], F32, tag="bps")
                    for k in range(CK):
                        nc.tensor.matmul(
                            ps[:], WQ[k][:, p * P:(p + 1) * P],
                            XT[k][:, n * 512:(n + 1) * 512],
                            start=(k == 0), stop=(k == CK - 1),
                        )
                    nc.vector.tensor_scalar_add(
                        QTs[p][:, n * 512:(n + 1) * 512], ps[:], bq_e[p][:])
                    ps = bdpsp.tile([P, 512], F32, tag="bps")
                    for k in range(CK):
                        nc.tensor.matmul(
                            ps[:], WK[k][:, p * P:(p + 1) * P],
                            XT[k][:, n * 512:(n + 1) * 512],
                            start=(k == 0), stop=(k == CK - 1),
                        )
                    nc.vector.tensor_scalar_add(
                        KTs[p][:, n * 512:(n + 1) * 512], ps[:], bk_e[p][:])

                def d_group(p, s):
                    # attention for BOTH heads of pair p, q super-block s.
                    # S^T chunks for h0/h1 run as concurrent PE row-tiles.
                    QT, KT = QTs[p], KTs[p]
                    nch = 4 * s + 4
                    av0 = avpool.tile([P, 512], F32, tag="av0")
                    av1 = avpool.tile([P, 512], F32, tag="av1")
                    pts = []

                    def emit_s(j):
                        sp = spool.tile([P, 1024], F32, tag="sp")
                        nc.tensor.matmul(
                            sp[:, 0:512],
                            KT[0:64, j * P:(j + 1) * P],
                            QT[0:64, s * 512:(s + 1) * 512],
                            start=True, stop=True,
                        )
                        nc.tensor.matmul(
                            sp[:, 512:1024],
                            KT[64:128, j * P:(j + 1) * P],
                            QT[64:128, s * 512:(s + 1) * 512],
                            start=True, stop=True,
                        )
                        pt = ptpool.tile([P, 1024], BF16, tag="pt")
                        nc.scalar.activation(pt[:], sp[:], AF.Exp, scale=0.125)
                        r = j - 4 * s
                        if 0 <= r <= 3:
                            nc.vector.tensor_mul(
                                pt[:], pt[:],
                                mask_t[:, r * 1024:(r + 1) * 1024])
                        pts.append(pt)

                    def emit_av(j):
                        pt = pts[j]
                        nc.tensor.matmul(
                            av0[:], V[j][:, 192 * p:192 * p + 128],
                            pt[:, 0:512],
                            start=(j == 0), stop=(j == nch - 1),
                        )
                        nc.tensor.matmul(
                            av1[:], V[j][:, 192 * p + 64:192 * p + 192],
                            pt[:, 512:1024],
                            start=(j == 0), stop=(j == nch - 1),
                        )

                    emit_s(0)
                    for j in range(1, nch):
                        emit_s(j)
                        emit_av(j - 1)
                    emit_av(nch - 1)

                    # normalization: reciprocal of denominators (h0 in av0
                    # rows 64:128, h1 in av1 rows 0:64), DMA partition-shift
                    # to the O^T rows, multiply into YT (bf16).
                    rec = lpool.tile([P, 512], BF16, tag="rec")
                    with nc.allow_low_precision(reason="softmax denom recip bf16"):
                        nc.vector.reciprocal(rec[64:128, :], av0[64:128, :])
                        nc.vector.reciprocal(rec[0:64, :], av1[0:64, :])
                    rec2 = lpool.tile([P, 512], BF16, tag="rec2")
                    nc.sync.dma_start(rec2[0:64, :], rec[64:128, :])
                    nc.sync.dma_start(rec2[64:128, :], rec[0:64, :])
                    nc.vector.tensor_mul(
                        YT[p][0:64, s * 512:(s + 1) * 512],
                        av0[0:64, :], rec2[0:64, :])
                    nc.vector.tensor_mul(
                        YT[p][64:128, s * 512:(s + 1) * 512],
                        av1[64:128, :], rec2[64:128, :])

                # emission: B(0) fully; pairs 0..2 interleaved with the next
                # pair's projection; pair 3 interleaved with the output
                # projection (bdps closed first to free PSUM banks for it).
                for n in range(4):
                    b_step(0, n)
                for p in range(3):
                    for s in range(NS):
                        d_group(p, s)
                        b_step(p + 1, s)
                bdps.__exit__(None, None, None)

                prpool_cm = tc.tile_pool(name="projps", bufs=2, space="PSUM")
                prpool = prpool_cm.__enter__()
                opool_cm = tc.tile_pool(name="ostage", bufs=4)
                opool = opool_cm.__enter__()
                last_ost = None

                def e_chunk(m):
                    nonlocal last_ost
                    for nn in range(2):
                        ps = prpool.tile([P, 512], F32, tag="pp")
                        for kf in range(4):
                            nc.tensor.matmul(
                                ps[:],
                                YT[kf][:, m * P:(m + 1) * P],
                                WP[kf][:, nn * 512:(nn + 1) * 512],
                                start=(kf == 0), stop=(kf == 3),
                            )
                        ost = opool.tile([P, 512], BF16, tag="ost")
                        if nn == 0:
                            nc.vector.tensor_copy(ost[:], ps[:])
                        else:
                            nc.scalar.copy(ost[:], ps[:])
                        nc.sync.dma_start(
                            out[m * P:(m + 1) * P, nn * 512:(nn + 1) * 512], ost[:])
                        last_ost = ost

                for s in range(NS):
                    d_group(3, s)
                    for m in range(4 * s, 4 * s + 4):
                        e_chunk(m)

                if chain:
                    tok = pp.tile([P, 1], F32, name=f"tok{_rep}", tag=f"tok{_rep}")
                    nc.vector.tensor_scalar_mul(tok[:], last_ost[:, 0:1], 0.0)
                if timing and _rep == repeat - 1:
                    nc.sync.dma_start(probe[:], last_ost[:, 0:4])

                opool_cm.__exit__(None, None, None)
                prpool_cm.__exit__(None, None, None)
                lpool_cm.__exit__(None, None, None)
                ptpool_cm.__exit__(None, None, None)
                avpool_cm.__exit__(None, None, None)
                spool_cm.__exit__(None, None, None)
                qtpool.__exit__(None, None, None)
                bd.__exit__(None, None, None)
                rep_pool_cm.__exit__(None, None, None)

    nc.compile()
    return nc


def _build_maskm():
    m = np.zeros((P, 4 * 1024), dtype=np.float32)
    k = np.arange(P)[:, None]
    q = np.arange(512)[None, :]
    for r in range(4):
        mr = (q >= 128 * r + k).astype(np.float32)
        m[:, r * 1024:r * 1024 + 512] = mr
        m[:, r * 1024 + 512:(r + 1) * 1024] = mr
    return m.astype(NPBF16)


_MASKM = _build_maskm()


def shard_inputs(x, w_attn, b_attn, w_proj, b_proj):
    """Build the per-core input maps (bf16 device inputs)."""
    x = np.asarray(x, dtype=np.float32)
    w_attn = np.asarray(w_attn, dtype=np.float32)
    b_attn = np.asarray(b_attn, dtype=np.float32)
    w_proj = np.asarray(w_proj, dtype=np.float32)
    in_maps = []
    for c in range(N_CORES):
        b, g = divmod(c, 2)
        sl = slice(g * 512, (g + 1) * 512)
        in_maps.append({
            "xt": np.ascontiguousarray(x[b].T).astype(NPBF16),
            "wq": np.ascontiguousarray(w_attn[:, g * 512:(g + 1) * 512]).astype(NPBF16),
            "wk": np.ascontiguousarray(w_attn[:, 1024 + g * 512:1024 + (g + 1) * 512]).astype(NPBF16),
            "wv": np.ascontiguousarray(w_attn[:, 2048 + g * 512:2048 + (g + 1) * 512]).astype(NPBF16),
            "wp": np.ascontiguousarray(w_proj[g * 512:(g + 1) * 512, :]).astype(NPBF16),
            "bq": np.ascontiguousarray(b_attn[sl].reshape(512, 1)),
            "bk": np.ascontiguousarray(b_attn[1024 + g * 512:1024 + (g + 1) * 512].reshape(512, 1)),
            "bv": np.ascontiguousarray(b_attn[2048 + g * 512:2048 + (g + 1) * 512].reshape(1, 512)).astype(NPBF16),
            "maskm": _MASKM,
        })
    return in_maps


def gather_output(results, b_proj):
    b_proj = np.asarray(b_proj, dtype=np.float32)
    out = np.empty((4, T, C), dtype=np.float32)
    for b in range(4):
        out[b] = (results[2 * b]["out"].astype(np.float32)
                  + results[2 * b + 1]["out"].astype(np.float32) + b_proj)
    return out


_NC_CACHE = None


def get_nc():
    global _NC_CACHE
    if _NC_CACHE is None:
        _NC_CACHE = build_nc()
    return _NC_CACHE


def kernel(x, w_attn, b_attn, w_proj, b_proj):
    from concourse.bass_utils import run_bass_kernel_spmd

    in_maps = shard_inputs(x, w_attn, b_attn, w_proj, b_proj)
    nc = get_nc()
    res = run_bass_kernel_spmd(nc, in_maps, list(range(N_CORES)))
    return gather_output(res.results, b_proj)
